# revision 8
# baseline (speedup 1.0000x reference)
"""DCE depth-classification loss on 8 Trainium2 NeuronCores.

Contract: kernel(**inputs) takes FULL inputs (target [4,1,256,384] f32,
mask [4,1,256,384] bool, pred_logit [4,200,256,384] f32,
bins_weight [200,200] f32) and returns the FULL scalar loss (np.float32).

Sharding: data-parallel over (batch, image-half): core k handles
b = k//2, rows h in [128*(k%2), 128*(k%2)+128) -> 49152 pixels/core.

Per-core math (pixels p, channels c in [0,200)):
  g_p   = depth bin of target (int in [0,199], sentinel 300 if invalid)
  lse_p = log(sum_c exp(x[c,p]))
  loss_sum = sum_p v_p * ( R(g_p)*lse_p - sum_c W[g_p,c]*x[c,p] )
where R(j) = sum_c W[j,c] (analytic: Rinf - 3-term Gaussian tails).
The data-heavy term is accumulated on the PE as a tiny matrix
  M[c,j] = sum_p x[c,p]*onehot(g_p==j)   (PSUM-accumulated over all pixels)
so that sum_p v*sum_c W[g_p,c] x[c,p] = sum_{c,j} W[j,c]*M[c,j], evaluated
on the host with the exact fp32 bins_weight.  bins_weight itself never
needs to go to the device.

Pipeline per 128-pixel batch: DMA fp32 -> GPSIMD cast bf16 -> PE transpose
(bf16, PSUM) -> ACT exp+accum (LSE) from PSUM / DVE evac -> DVE onehot
(is_equal vs iota, bf16 4x) -> 2 PE matmuls accumulating M.
"""

import math
from contextlib import ExitStack

import numpy as np

import concourse.bacc as bacc
import concourse.bass as bass
import concourse.mybir as mybir
import concourse.tile as tile
from concourse import bass_utils

F32 = mybir.dt.float32
BF16 = mybir.dt.bfloat16
I32 = mybir.dt.int32
U8 = mybir.dt.uint8
AF = mybir.ActivationFunctionType
OP = mybir.AluOpType
AX = mybir.AxisListType

BINS = 200
DEPTH_MIN = 1.0
DEPTH_MAX = 80.0
ALPHA = 2.0
EPS = 1e-6
B, H, W = 4, 256, 384
N_CORES = 8
HH = H // 2              # 128 rows per core
PIX = HH * W             # 49152 pixels per core
NB = PIX // 128          # 384 batches of 128 pixels
NSB = 24                 # super-blocks (DMA granularity)
BPS = NB // NSB          # 16 batches / super-block
SBW = BPS * 128          # 2048 pixels / super-block
GRP = 5                  # batches per PSUM transpose tile (5*200*2B = 2KB bank)
SENT = 300.0             # invalid-pixel bin sentinel (exact in bf16, != 0..199)

# bins_f = log10(d)/INTERVAL = ln(d) * (200/ln(80))
LOG_SCALE = 200.0 / math.log(80.0)
# R(g) = RINF - T(g) - T(199-g), T(m) = sum_{d>m} exp(-2 d^2) (3 terms suffice)
RINF = 1.0 + 2.0 * sum(math.exp(-2.0 * d * d) for d in range(1, 8))


def _body(ctx: ExitStack, tc: "tile.TileContext", x_ap, tgt_ap, msk_ap,
          m_out_ap, misc_ap):
    nc = tc.nc
    const = ctx.enter_context(tc.tile_pool(name="const", bufs=1))
    sb = ctx.enter_context(tc.tile_pool(name="sb", bufs=3))
    sbb = ctx.enter_context(tc.tile_pool(name="sbb", bufs=3))
    xtp = ctx.enter_context(tc.tile_pool(name="xtp", bufs=3))
    ohp = ctx.enter_context(tc.tile_pool(name="ohp", bufs=3))
    scr = ctx.enter_context(tc.tile_pool(name="scr", bufs=2))
    smal = ctx.enter_context(tc.tile_pool(name="smal", bufs=2))
    psum = ctx.enter_context(tc.tile_pool(name="psum", bufs=2, space="PSUM"))
    gps = ctx.enter_context(tc.tile_pool(name="gps", bufs=1, space="PSUM"))
    mps = ctx.enter_context(tc.tile_pool(name="mps", bufs=1, space="PSUM"))

    # ---- constants ----
    iota_i = const.tile([128, BINS], I32)
    nc.gpsimd.iota(iota_i[:], pattern=[[1, BINS]], base=0, channel_multiplier=0)
    iota_bf = const.tile([128, BINS], BF16)
    nc.vector.tensor_copy(iota_bf[:], iota_i[:])

    diag_i = const.tile([128, 128], I32)
    nc.gpsimd.iota(diag_i[:], pattern=[[1, 128]], base=0, channel_multiplier=-1)
    ident = const.tile([128, 128], BF16)
    nc.vector.tensor_scalar(ident[:], diag_i[:], 0, None, OP.is_equal)

    zeros = const.tile([128, W], F32)
    nc.vector.memset(zeros[:], 0.0)
    c199 = const.tile([128, W], F32)
    nc.vector.memset(c199[:], 199.0)
    c300 = const.tile([128, W], F32)
    nc.vector.memset(c300[:], SENT)

    # persistent small state
    S_T = const.tile([128, NB], F32)      # per-pixel sum(exp), transposed order
    misc = const.tile([128, 2], F32)      # [:,0]=sum R*lse*v, [:,1]=count
    M1 = mps.tile([128, BINS], F32, tag="m1")   # M[c,j] c in [0,128)
    M2 = mps.tile([72, BINS], F32, tag="m2")    # M[c,j] c in [128,200)

    # ---- prologue: bins g, valid v (natural [128h, 384w] layout) ----
    tgt = smal.tile([128, W], F32, tag="tgt")
    nc.sync.dma_start(tgt[:], tgt_ap[:, :])
    msk = smal.tile([128, W], U8, tag="msk")
    nc.sync.dma_start(msk[:], msk_ap[:, :])

    v_f = smal.tile([128, W], F32, tag="vf")
    nc.vector.tensor_copy(v_f[:], msk[:])
    a_t = smal.tile([128, W], F32, tag="at")
    nc.scalar.activation(a_t[:], tgt[:], AF.Abs)
    l_t = smal.tile([128, W], F32, tag="lt")
    nc.scalar.activation(l_t[:], a_t[:], AF.Ln)
    bf_t = smal.tile([128, W], F32, tag="bft")
    # bins_f = max(ln(a)*LOG_SCALE, -1)  (clamp kills -inf from a==0)
    nc.vector.tensor_scalar(bf_t[:], l_t[:], LOG_SCALE, -1.0, OP.mult, OP.max)
    t_i = smal.tile([128, W], I32, tag="ti")
    nc.vector.tensor_copy(t_i[:], bf_t[:])
    t_f = smal.tile([128, W], F32, tag="tf")
    nc.vector.tensor_copy(t_f[:], t_i[:])
    # floor fix: if cast rounded up, subtract 1
    gt_m = smal.tile([128, W], F32, tag="gtm")
    nc.vector.tensor_tensor(gt_m[:], t_f[:], bf_t[:], OP.is_gt)
    nc.vector.tensor_tensor(t_f[:], t_f[:], gt_m[:], OP.subtract)
    nc.vector.tensor_scalar(t_f[:], t_f[:], 199.0, None, OP.min)
    # clamps + sentinel
    m0 = smal.tile([128, W], I32, tag="m0")
    nc.vector.tensor_scalar(m0[:], a_t[:], DEPTH_MIN, None, OP.is_le)
    m199 = smal.tile([128, W], I32, tag="m199")
    nc.vector.tensor_scalar(m199[:], a_t[:], DEPTH_MAX, None, OP.is_ge)
    minv = smal.tile([128, W], I32, tag="minv")
    nc.vector.tensor_scalar(minv[:], v_f[:], 0.5, None, OP.is_le)
    g_f = smal.tile([128, W], F32, tag="gf")
    nc.vector.select(g_f[:], m0[:], zeros[:], t_f[:])
    nc.vector.copy_predicated(g_f[:], m199[:], c199[:])
    nc.vector.copy_predicated(g_f[:], minv[:], c300[:])
    g_bf = smal.tile([128, W], BF16, tag="gbf")
    nc.vector.tensor_copy(g_bf[:], g_f[:])
    v_bf = smal.tile([128, W], BF16, tag="vbf")
    nc.vector.tensor_copy(v_bf[:], v_f[:])

    # transpose g and v: [128h, 3*128w] -> [128w, (third, h)]
    gv_ps = gps.tile([128, 2 * W], BF16, tag="gvps")
    for t in range(3):
        nc.tensor.transpose(gv_ps[:, bass.ts(t, 128)], g_bf[:, bass.ts(t, 128)],
                            ident[:])
        nc.tensor.transpose(gv_ps[:, W + t * 128:W + (t + 1) * 128],
                            v_bf[:, bass.ts(t, 128)], ident[:])
    gT = const.tile([128, W], F32)
    nc.vector.tensor_copy(gT[:], gv_ps[:, 0:W])
    vT = const.tile([128, W], BF16)
    nc.vector.tensor_copy(vT[:], gv_ps[:, W:2 * W])

    def g_col(b):
        # batch b covers pixels [128b, 128b+128): h = b//3, third = b%3
        return gT[:, (b % 3) * 128 + (b // 3):(b % 3) * 128 + (b // 3) + 1]

    # ---- main loop ----
    for sbi in range(NSB):
        px = sbi * SBW
        xa = sb.tile([128, SBW], F32, tag="xa")
        nc.sync.dma_start(xa[:], x_ap[0:128, px:px + SBW])
        xb = sb.tile([72, SBW], F32, tag="xb")
        nc.sync.dma_start(xb[:], x_ap[128:200, px:px + SBW])
        xab = sbb.tile([128, SBW], BF16, tag="xab")
        nc.gpsimd.tensor_copy(xab[:], xa[:])
        xbb = sbb.tile([72, SBW], BF16, tag="xbb")
        nc.gpsimd.tensor_copy(xbb[:], xb[:])

        for gi in range(BPS // GRP + (1 if BPS % GRP else 0)):
            n_in_g = min(GRP, BPS - gi * GRP)
            xt_ps = psum.tile([128, GRP * BINS], BF16, tag="xtps")
            for t in range(n_in_g):
                tt = gi * GRP + t
                b = sbi * BPS + tt
                nc.tensor.transpose(
                    xt_ps[:, t * BINS:t * BINS + 128],
                    xab[:, bass.ts(tt, 128)], ident[:])
                nc.tensor.transpose(
                    xt_ps[:, t * BINS + 128:t * BINS + 200],
                    xbb[:, bass.ts(tt, 128)], ident[0:72, 0:72])
            # LSE partial: exp + free-dim accumulate, straight from PSUM
            for t in range(n_in_g):
                b = sbi * BPS + gi * GRP + t
                e_scr = scr.tile([128, BINS], BF16, tag="escr")
                nc.scalar.activation(e_scr[:], xt_ps[:, t * BINS:(t + 1) * BINS],
                                     AF.Exp, accum_out=S_T[:, b:b + 1])
            # evacuate whole transpose group to SBUF for the PE
            xt_sb = xtp.tile([128, GRP * BINS], BF16, tag="xtsb")
            nc.vector.tensor_copy(xt_sb[:, 0:n_in_g * BINS],
                                  xt_ps[:, 0:n_in_g * BINS])
            for t in range(n_in_g):
                b = sbi * BPS + gi * GRP + t
                oh = ohp.tile([128, BINS], BF16, tag="oh")
                nc.vector.tensor_scalar(oh[:], iota_bf[:], g_col(b), None,
                                        OP.is_equal)
                nc.tensor.matmul(M1[:], xt_sb[:, t * BINS:t * BINS + 128],
                                 oh[:], start=(b == 0), stop=(b == NB - 1),
                                 skip_group_check=True)
                nc.tensor.matmul(M2[:], xt_sb[:, t * BINS + 128:t * BINS + 200],
                                 oh[:], start=(b == 0), stop=(b == NB - 1),
                                 skip_group_check=True)

    # ---- epilogue ----
    lse = smal.tile([128, NB], F32, tag="lse")
    nc.scalar.activation(lse[:], S_T[:], AF.Ln)
    # R(g) = RINF - sum_k exp(-2 (g+k)^2) - sum_k exp(-2 (199+k-g)^2), k=1..3
    bias6 = const.tile([128, 6], F32)
    for i, bv in enumerate((1.0, 2.0, 3.0, 200.0, 201.0, 202.0)):
        nc.vector.memset(bias6[:, i:i + 1], bv)
    R_t = smal.tile([128, NB], F32, tag="rt")
    nc.vector.memset(R_t[:], RINF)
    for i, k in enumerate((1, 2, 3)):
        sq = smal.tile([128, NB], F32, tag="sq")
        nc.scalar.activation(sq[:], gT[:], AF.Square, bias=bias6[:, i:i + 1],
                             scale=1.0)
        term = smal.tile([128, NB], F32, tag="term")
        nc.scalar.activation(term[:], sq[:], AF.Exp, scale=-ALPHA)
        nc.vector.tensor_tensor(R_t[:], R_t[:], term[:], OP.subtract)
        nc.scalar.activation(sq[:], gT[:], AF.Square,
                             bias=bias6[:, 3 + i:4 + i], scale=-1.0)
        nc.scalar.activation(term[:], sq[:], AF.Exp, scale=-ALPHA)
        nc.vector.tensor_tensor(R_t[:], R_t[:], term[:], OP.subtract)
    # P1 = sum_b R*lse*v ; count = sum v
    p1 = smal.tile([128, NB], F32, tag="p1")
    nc.vector.tensor_tensor(p1[:], R_t[:], lse[:], OP.mult)
    nc.vector.tensor_tensor(p1[:], p1[:], vT[:], OP.mult)
    p1s = smal.tile([128, NB], F32, tag="p1s")
    nc.vector.tensor_scalar(p1s[:], p1[:], 1.0, None, OP.mult, OP.add,
                            accum_out=misc[:, 0:1])
    cnt = smal.tile([128, NB], F32, tag="cnt")
    nc.vector.tensor_scalar(cnt[:], vT[:], 1.0, None, OP.mult, OP.add,
                            accum_out=misc[:, 1:2])

    # M -> DRAM
    m_sb1 = smal.tile([128, BINS], F32, tag="msb1")
    nc.vector.tensor_copy(m_sb1[:], M1[:])
    m_sb2 = smal.tile([72, BINS], F32, tag="msb2")
    nc.vector.tensor_copy(m_sb2[:], M2[:])
    nc.sync.dma_start(m_out_ap[0:128, :], m_sb1[:])
    nc.sync.dma_start(m_out_ap[128:200, :], m_sb2[:])
    nc.sync.dma_start(misc_ap[:, :], misc[:])


_CACHE = {}


def build_nc():
    if "nc" in _CACHE:
        return _CACHE["nc"]
    nc = bacc.Bacc("TRN2", target_bir_lowering=False, debug=False)
    x = nc.dram_tensor("x", [BINS, PIX], F32, kind="ExternalInput")
    tgt = nc.dram_tensor("tgt", [HH, W], F32, kind="ExternalInput")
    msk = nc.dram_tensor("msk", [HH, W], U8, kind="ExternalInput")
    m_out = nc.dram_tensor("m_out", [BINS, BINS], F32, kind="ExternalOutput")
    misc = nc.dram_tensor("misc_out", [128, 2], F32, kind="ExternalOutput")
    with tile.TileContext(nc) as tc:
        with ExitStack() as ctx:
            _body(ctx, tc, x.ap(), tgt.ap(), msk.ap(), m_out.ap(), misc.ap())
    nc.compile()
    _CACHE["nc"] = nc
    return nc


def shard_inputs(target, mask, pred_logit):
    """Per-core input dicts. Core k: b = k//2, rows [128*(k%2), +128)."""
    in_maps = []
    for k in range(N_CORES):
        b, hh = k // 2, (k % 2) * HH
        in_maps.append({
            "x": np.ascontiguousarray(
                pred_logit[b, :, hh:hh + HH, :].reshape(BINS, PIX)),
            "tgt": np.ascontiguousarray(target[b, 0, hh:hh + HH, :]),
            "msk": np.ascontiguousarray(
                mask[b, 0, hh:hh + HH, :]).astype(np.uint8),
        })
    return in_maps


def combine(results, bins_weight):
    """Host-side reduction of per-core outputs -> scalar loss."""
    Wm = bins_weight.astype(np.float64)
    m_tot = np.zeros((BINS, BINS), np.float64)
    p1_tot = 0.0
    cnt_tot = 0.0
    for r in results:
        m_tot += r["m_out"].astype(np.float64)
        p1_tot += float(r["misc_out"][:, 0].sum(dtype=np.float64))
        cnt_tot += float(r["misc_out"][:, 1].sum(dtype=np.float64))
    # sum_{c,j} W[j,c] * M[c,j]  (W symmetric)
    dot_tot = float((Wm * m_tot).sum())
    loss = (p1_tot - dot_tot) / (cnt_tot + EPS)
    return np.float32(loss)


def kernel(target, mask, pred_logit, bins_weight):
    target = np.asarray(target, dtype=np.float32)
    mask = np.asarray(mask)
    pred_logit = np.asarray(pred_logit, dtype=np.float32)
    bins_weight = np.asarray(bins_weight, dtype=np.float32)
    nc = build_nc()
    in_maps = shard_inputs(target, mask, pred_logit)
    res = bass_utils.run_bass_kernel_spmd(nc, in_maps,
                                          core_ids=list(range(N_CORES)))
    return combine(res.results, bins_weight)


if __name__ == "__main__":
    np.random.seed(0)
    tgt = (np.random.rand(B, 1, H, W) * 100).astype(np.float32)
    msk = np.random.rand(B, 1, H, W) > 0.1
    x = np.random.randn(B, BINS, H, W).astype(np.float32)
    idx = np.arange(BINS, dtype=np.float32)
    wb = np.exp(-ALPHA * (idx[None, :] - idx[:, None]) ** 2).astype(np.float32)
    print(kernel(tgt, msk, x, wb))


# revision 26
# speedup vs baseline: 274.3726x; 274.3726x over previous
"""DCE depth-classification loss on 8 Trainium2 NeuronCores.

Contract: kernel(**inputs) takes FULL inputs (target [4,1,256,384] f32,
mask [4,1,256,384] bool, pred_logit [4,200,256,384] f32,
bins_weight [200,200] f32) and returns the FULL scalar loss (np.float32).

Sharding: data-parallel over (batch, image-half): core k handles
b = k//2, rows h in [128*(k%2), 128*(k%2)+128) -> 49152 pixels/core.

Per-core math (pixels p, channels c in [0,200)):
  g_p   = depth bin of target (int in [0,199], sentinel 300 if invalid)
  lse_p = log(sum_c exp(x[c,p]))
  loss_sum = sum_p v_p * ( R(g_p)*lse_p - sum_c W[g_p,c]*x[c,p] )
where R(j) = sum_c W[j,c] (analytic: Rinf - 3-term Gaussian tails).
The data-heavy term is accumulated on the PE as a tiny matrix
  M[c,j] = sum_p x[c,p]*onehot(g_p==j)   (PSUM-accumulated over all pixels)
so that sum_p v*sum_c W[g_p,c] x[c,p] = sum_{c,j} W[j,c]*M[c,j], evaluated
on the host with the exact fp32 bins_weight.  bins_weight itself never
needs to go to the device.

Pipeline per 128-pixel batch: DMA fp32 -> GPSIMD cast bf16 -> PE transpose
(bf16, PSUM) -> ACT exp+accum (LSE) from PSUM / DVE evac -> DVE onehot
(is_equal vs iota, bf16 4x) -> 2 PE matmuls accumulating M.
"""

import math
from contextlib import ExitStack

import numpy as np

import concourse.bacc as bacc
import concourse.bass as bass
import concourse.mybir as mybir
import concourse.tile as tile
from concourse import bass_utils

F32 = mybir.dt.float32
BF16 = mybir.dt.bfloat16
I32 = mybir.dt.int32
U8 = mybir.dt.uint8
AF = mybir.ActivationFunctionType
OP = mybir.AluOpType
AX = mybir.AxisListType

BINS = 200
DEPTH_MIN = 1.0
DEPTH_MAX = 80.0
ALPHA = 2.0
EPS = 1e-6
B, H, W = 4, 256, 384
N_CORES = 8
HH = H // 2              # 128 rows per core
PIX = HH * W             # 49152 pixels per core
NB = PIX // 128          # 384 batches of 128 pixels
NSB = 24                 # super-blocks (DMA granularity)
BPS = NB // NSB          # 16 batches / super-block
SBW = BPS * 128          # 2048 pixels / super-block
CA = 128                 # first c-chunk
CBR = 72                 # real rows in second chunk
CB = 80                  # padded to %16==0 for xbar transpose
BSTR = CA + CB           # 208: per-batch column stride in transposed tiles
GRP = 4                  # batches per PSUM transpose tile (5*200*2B = 2KB bank)
SENT = 300.0             # invalid-pixel bin sentinel (exact in bf16, != 0..199)

# bins_f = log10(d)/INTERVAL = ln(d) * (200/ln(80))
LOG_SCALE = 200.0 / math.log(80.0)
# R(g) = RINF - T(g) - T(199-g), T(m) = sum_{d>m} exp(-2 d^2) (3 terms suffice)
RINF = 1.0 + 2.0 * sum(math.exp(-2.0 * d * d) for d in range(1, 8))


def _body(ctx: ExitStack, tc: "tile.TileContext", x_ap, tgt_ap, msk_ap,
          m_out_ap, misc_ap):
    nc = tc.nc
    const = ctx.enter_context(tc.tile_pool(name="const", bufs=1))
    sb = ctx.enter_context(tc.tile_pool(name="sb", bufs=4))
    sbb = ctx.enter_context(tc.tile_pool(name="sbb", bufs=5))
    xtp = ctx.enter_context(tc.tile_pool(name="xtp", bufs=4))
    ohp = ctx.enter_context(tc.tile_pool(name="ohp", bufs=6))
    scr = ctx.enter_context(tc.tile_pool(name="scr", bufs=4))
    smal = ctx.enter_context(tc.tile_pool(name="smal", bufs=2))
    psum = ctx.enter_context(tc.tile_pool(name="psum", bufs=4, space="PSUM"))
    gps = ctx.enter_context(tc.tile_pool(name="gps", bufs=1, space="PSUM"))
    mps = ctx.enter_context(tc.tile_pool(name="mps", bufs=1, space="PSUM"))

    # ---- constants ----
    iota_i = const.tile([128, BINS], I32)
    nc.gpsimd.iota(iota_i[:], pattern=[[1, BINS]], base=0, channel_multiplier=0)
    iota_bf = const.tile([128, BINS], BF16)
    nc.vector.tensor_copy(iota_bf[:], iota_i[:])

    diag_i = const.tile([128, 128], I32)
    nc.gpsimd.iota(diag_i[:], pattern=[[1, 128]], base=0, channel_multiplier=-1)
    ident = const.tile([128, 128], BF16)
    nc.vector.tensor_scalar(ident[:], diag_i[:], 0, None, OP.is_equal)

    zeros = const.tile([128, W], F32)
    nc.vector.memset(zeros[:], 0.0)
    c199 = const.tile([128, W], F32)
    nc.vector.memset(c199[:], 199.0)
    c300 = const.tile([128, W], F32)
    nc.vector.memset(c300[:], SENT)

    # persistent small state
    S_T = const.tile([128, NB], F32)      # per-pixel sum(exp), transposed order
    if "exp" in ABLATE:
        nc.vector.memset(S_T[:], 1.0)
    misc = const.tile([128, 2], F32)      # [:,0]=sum R*lse*v, [:,1]=count
    if "mm" not in ABLATE:
        M1 = mps.tile([CA, BINS], F32, tag="m1")    # M[c,j] c in [0,128)
        M2 = mps.tile([CBR, BINS], F32, tag="m2")   # M[c,j] c in [128,200)
    else:
        M1 = M2 = None

    # ---- prologue: bins g, valid v (natural [128h, 384w] layout) ----
    tgt = smal.tile([128, W], F32, tag="tgt")
    nc.sync.dma_start(tgt[:], tgt_ap[:, :])
    msk = smal.tile([128, W], U8, tag="msk")
    nc.sync.dma_start(msk[:], msk_ap[:, :])

    v_f = smal.tile([128, W], F32, tag="vf")
    nc.vector.tensor_copy(v_f[:], msk[:])
    a_t = smal.tile([128, W], F32, tag="at")
    nc.scalar.activation(a_t[:], tgt[:], AF.Abs)
    l_t = smal.tile([128, W], F32, tag="lt")
    nc.scalar.activation(l_t[:], a_t[:], AF.Ln)
    bf_t = smal.tile([128, W], F32, tag="bft")
    # bins_f = max(ln(a)*LOG_SCALE, -1)  (clamp kills -inf from a==0)
    nc.vector.tensor_scalar(bf_t[:], l_t[:], LOG_SCALE, -1.0, OP.mult, OP.max)
    t_i = smal.tile([128, W], I32, tag="ti")
    nc.vector.tensor_copy(t_i[:], bf_t[:])
    t_f = smal.tile([128, W], F32, tag="tf")
    nc.vector.tensor_copy(t_f[:], t_i[:])
    # floor fix: if cast rounded up, subtract 1
    gt_m = smal.tile([128, W], F32, tag="gtm")
    nc.vector.tensor_tensor(gt_m[:], t_f[:], bf_t[:], OP.is_gt)
    nc.vector.tensor_tensor(t_f[:], t_f[:], gt_m[:], OP.subtract)
    nc.vector.tensor_scalar(t_f[:], t_f[:], 199.0, None, OP.min)
    # clamps + sentinel
    m0 = smal.tile([128, W], I32, tag="m0")
    nc.vector.tensor_scalar(m0[:], a_t[:], DEPTH_MIN, None, OP.is_le)
    m199 = smal.tile([128, W], I32, tag="m199")
    nc.vector.tensor_scalar(m199[:], a_t[:], DEPTH_MAX, None, OP.is_ge)
    minv = smal.tile([128, W], I32, tag="minv")
    nc.vector.tensor_scalar(minv[:], v_f[:], 0.5, None, OP.is_le)
    g_f = smal.tile([128, W], F32, tag="gf")
    nc.vector.select(g_f[:], m0[:], zeros[:], t_f[:])
    nc.vector.copy_predicated(g_f[:], m199[:], c199[:])
    nc.vector.copy_predicated(g_f[:], minv[:], c300[:])
    g_bf = smal.tile([128, W], BF16, tag="gbf")
    nc.vector.tensor_copy(g_bf[:], g_f[:])
    v_bf = smal.tile([128, W], BF16, tag="vbf")
    nc.vector.tensor_copy(v_bf[:], v_f[:])

    # transpose g and v: [128h, 3*128w] -> [128w, (third, h)]
    gv_ps = gps.tile([128, 2 * W], BF16, tag="gvps")
    for t in range(3):
        nc.tensor.transpose(gv_ps[:, bass.ts(t, 128)], g_bf[:, bass.ts(t, 128)],
                            ident[:])
        nc.tensor.transpose(gv_ps[:, W + t * 128:W + (t + 1) * 128],
                            v_bf[:, bass.ts(t, 128)], ident[:])
    gT = const.tile([128, W], F32)
    nc.vector.tensor_copy(gT[:], gv_ps[:, 0:W])
    vT = const.tile([128, W], BF16)
    nc.vector.tensor_copy(vT[:], gv_ps[:, W:2 * W])

    def g_col(b):
        # batch b covers pixels [128b, 128b+128): h = b//3, third = b%3
        return gT[:, (b % 3) * 128 + (b // 3):(b % 3) * 128 + (b // 3) + 1]

    # ---- main loop (1-group software pipeline skew) ----
    pending = []

    def flush_pending():
        for fn in pending:
            fn()
        pending.clear()

    for sbi in range(NSB):
        px = sbi * SBW
        xa = sb.tile([CA, SBW], F32, tag="xa")
        xb = sb.tile([CB, SBW], F32, tag="xb")
        nc.vector.memset(xb[64:CB, :], 0.0)  # pad rows (start%32==0)
        if TRMODE == "dma":
            nc.gpsimd.dma_start(xa[:], x_ap[0:CA, px:px + SBW])
            nc.gpsimd.dma_start(xb[0:CBR, :], x_ap[CA:200, px:px + SBW])
        else:
            nc.sync.dma_start(xa[:], x_ap[0:CA, px:px + SBW])
            nc.sync.dma_start(xb[0:CBR, :], x_ap[CA:200, px:px + SBW])
        xab = sbb.tile([CA, SBW], BF16, tag="xab")
        xbb = sbb.tile([CB, SBW], BF16, tag="xbb")
        if "cast" in ABLATE:
            nc.vector.memset(xab[:], 0.0)
            nc.vector.memset(xbb[:], 0.0)
        else:
            hw_ = SBW // 2
            nc.gpsimd.tensor_copy(xab[:, 0:hw_], xa[:, 0:hw_])
            nc.gpsimd.tensor_copy(xab[:, hw_:SBW], xa[:, hw_:SBW])
            if sbi % 6 == 5:
                nc.vector.tensor_copy(xbb[:, 0:hw_], xb[:, 0:hw_])
                nc.vector.tensor_copy(xbb[:, hw_:SBW], xb[:, hw_:SBW])
            else:
                nc.gpsimd.tensor_copy(xbb[:, 0:hw_], xb[:, 0:hw_])
                nc.gpsimd.tensor_copy(xbb[:, hw_:SBW], xb[:, hw_:SBW])

        for gi in range(BPS // GRP + (1 if BPS % GRP else 0)):
            n_in_g = min(GRP, BPS - gi * GRP)
            b0 = sbi * BPS + gi * GRP
            if TRMODE == "dma":
                xt_ps = None
                xt_sb0 = xtp.tile([128, GRP * BSTR], BF16, tag="xtsb")
                for t in range(n_in_g):
                    if "tr" in ABLATE:
                        nc.vector.memset(xt_sb0[:], 0.0)
                        break
                    tt = gi * GRP + t
                    nc.sync.dma_start_transpose(
                        xt_sb0[:, t * BSTR:t * BSTR + CA],
                        xab[:, bass.ts(tt, 128)])
                    nc.sync.dma_start_transpose(
                        xt_sb0[:, t * BSTR + CA:(t + 1) * BSTR],
                        xbb[:, bass.ts(tt, 128)])
            else:
                xt_sb0 = None
                xt_ps = psum.tile([128, GRP * BSTR], BF16, tag="xtps")
                if "tr" in ABLATE and ("exp" not in ABLATE
                                       or "evac" not in ABLATE):
                    nc.vector.memset(xt_ps[:], 0.0)
                for t in range(n_in_g):
                    if "tr" in ABLATE:
                        break
                    tt = gi * GRP + t
                    nc.tensor.transpose(
                        xt_ps[:, t * BSTR:t * BSTR + CA],
                        xab[:, bass.ts(tt, 128)], ident[0:CA, 0:CA])
                    nc.tensor.transpose(
                        xt_ps[:, t * BSTR + CA:(t + 1) * BSTR],
                        xbb[:, bass.ts(tt, 128)], ident[0:CB, 0:CB])
            flush_pending()

            def consume(xt_ps=xt_ps, xt_sb0=xt_sb0, b0=b0, n_in_g=n_in_g):
                if TRMODE == "dma":
                    xt_sb = xt_sb0
                else:
                    # evacuate whole transpose group to SBUF (frees PSUM),
                    # split between DVE and ACT
                    xt_sb = xtp.tile([128, GRP * BSTR], BF16, tag="xtsb")
                    if "evac" not in ABLATE:
                        half = (n_in_g // 2) * BSTR
                        if half > 0:
                            nc.scalar.copy(xt_sb[:, 0:half], xt_ps[:, 0:half])
                        nc.vector.tensor_copy(xt_sb[:, half:n_in_g * BSTR],
                                              xt_ps[:, half:n_in_g * BSTR])
                    elif "mm" not in ABLATE or "exp" not in ABLATE:
                        nc.vector.memset(xt_sb[:], 0.0)
                # LSE: one big exp from SBUF -> E (bf16), then per-batch
                # free-dim sums on DVE (ts-accum runs in 4x mode)
                if "exp" not in ABLATE:
                    e_t = scr.tile([128, GRP * BSTR], BF16, tag="escr")
                    nc.scalar.activation(e_t[:, 0:n_in_g * BSTR],
                                         xt_sb[:, 0:n_in_g * BSTR], AF.Exp)
                    for t in range(n_in_g):
                        b = b0 + t
                        s_scr = scr.tile([128, BINS], BF16, tag="sscr")
                        nc.vector.tensor_scalar(
                            s_scr[:], e_t[:, t * BSTR:t * BSTR + BINS],
                            1.0, None, OP.mult, OP.add,
                            accum_out=S_T[:, b:b + 1])
                for t in range(n_in_g):
                    if "mm" in ABLATE:
                        break
                    b = b0 + t
                    oh = ohp.tile([128, BINS], BF16, tag="oh")
                    if "oh" in ABLATE:
                        nc.vector.memset(oh[:], 0.0)
                    else:
                        nc.vector.tensor_scalar(oh[:], iota_bf[:], g_col(b),
                                                None, OP.is_equal)
                    nc.tensor.matmul(M1[:], xt_sb[:, t * BSTR:t * BSTR + CA],
                                     oh[:], start=(b == 0), stop=(b == NB - 1),
                                     skip_group_check=True)
                    nc.tensor.matmul(M2[:],
                                     xt_sb[:, t * BSTR + CA:t * BSTR + 200],
                                     oh[:], start=(b == 0), stop=(b == NB - 1),
                                     skip_group_check=True)

            pending.append(consume)
    flush_pending()

    # ---- epilogue ----
    lse = smal.tile([128, NB], F32, tag="lse")
    nc.scalar.activation(lse[:], S_T[:], AF.Ln)
    # R(g) = RINF - sum_k exp(-2 (g+k)^2) - sum_k exp(-2 (199+k-g)^2), k=1..3
    bias6 = const.tile([128, 6], F32)
    for i, bv in enumerate((1.0, 2.0, 3.0, 200.0, 201.0, 202.0)):
        nc.vector.memset(bias6[:, i:i + 1], bv)
    R_t = smal.tile([128, NB], F32, tag="rt")
    nc.vector.memset(R_t[:], RINF)
    for i, k in enumerate((1, 2, 3)):
        sq = smal.tile([128, NB], F32, tag="sq")
        nc.scalar.activation(sq[:], gT[:], AF.Square, bias=bias6[:, i:i + 1],
                             scale=1.0)
        term = smal.tile([128, NB], F32, tag="term")
        nc.scalar.activation(term[:], sq[:], AF.Exp, scale=-ALPHA)
        nc.vector.tensor_tensor(R_t[:], R_t[:], term[:], OP.subtract)
        nc.scalar.activation(sq[:], gT[:], AF.Square,
                             bias=bias6[:, 3 + i:4 + i], scale=-1.0)
        nc.scalar.activation(term[:], sq[:], AF.Exp, scale=-ALPHA)
        nc.vector.tensor_tensor(R_t[:], R_t[:], term[:], OP.subtract)
    # P1 = sum_b R*lse*v ; count = sum v
    p1 = smal.tile([128, NB], F32, tag="p1")
    nc.vector.tensor_tensor(p1[:], R_t[:], lse[:], OP.mult)
    nc.vector.tensor_tensor(p1[:], p1[:], vT[:], OP.mult)
    p1s = smal.tile([128, NB], F32, tag="p1s")
    nc.vector.tensor_scalar(p1s[:], p1[:], 1.0, None, OP.mult, OP.add,
                            accum_out=misc[:, 0:1])
    cnt = smal.tile([128, NB], F32, tag="cnt")
    nc.vector.tensor_scalar(cnt[:], vT[:], 1.0, None, OP.mult, OP.add,
                            accum_out=misc[:, 1:2])

    # M -> DRAM
    m_sb1 = smal.tile([CA, BINS], F32, tag="msb1")
    m_sb2 = smal.tile([CBR, BINS], F32, tag="msb2")
    if "mm" in ABLATE:
        nc.vector.memset(m_sb1[:], 0.0)
        nc.vector.memset(m_sb2[:], 0.0)
    else:
        nc.vector.tensor_copy(m_sb1[:], M1[:])
        nc.vector.tensor_copy(m_sb2[:], M2[:])
    nc.sync.dma_start(m_out_ap[0:CA, :], m_sb1[:])
    nc.sync.dma_start(m_out_ap[CA:200, :], m_sb2[:])
    nc.sync.dma_start(misc_ap[:, :], misc[:])


ABLATE = set()
TRMODE = "pe"    # "pe" or "dma" transposes

_CACHE = {}


def build_nc(reps=1):
    key = (tuple(sorted(ABLATE)), reps)
    if key in _CACHE:
        return _CACHE[key]
    nc = bacc.Bacc("TRN2", target_bir_lowering=False, debug=False)
    x = nc.dram_tensor("x", [BINS, PIX], F32, kind="ExternalInput")
    tgt = nc.dram_tensor("tgt", [HH, W], F32, kind="ExternalInput")
    msk = nc.dram_tensor("msk", [HH, W], U8, kind="ExternalInput")
    m_out = nc.dram_tensor("m_out", [BINS, BINS], F32, kind="ExternalOutput")
    misc = nc.dram_tensor("misc_out", [128, 2], F32, kind="ExternalOutput")
    with tile.TileContext(nc) as tc:
        for _ in range(reps):
            with ExitStack() as ctx:
                _body(ctx, tc, x.ap(), tgt.ap(), msk.ap(), m_out.ap(),
                      misc.ap())
    nc.compile()
    _CACHE[key] = nc
    return nc


def shard_inputs(target, mask, pred_logit):
    """Per-core input dicts. Core k: b = k//2, rows [128*(k%2), +128)."""
    in_maps = []
    for k in range(N_CORES):
        b, hh = k // 2, (k % 2) * HH
        in_maps.append({
            "x": np.ascontiguousarray(
                pred_logit[b, :, hh:hh + HH, :].reshape(BINS, PIX)),
            "tgt": np.ascontiguousarray(target[b, 0, hh:hh + HH, :]),
            "msk": np.ascontiguousarray(
                mask[b, 0, hh:hh + HH, :]).astype(np.uint8),
        })
    return in_maps


def combine(results, bins_weight):
    """Host-side reduction of per-core outputs -> scalar loss."""
    Wm = bins_weight.astype(np.float64)
    m_tot = np.zeros((BINS, BINS), np.float64)
    p1_tot = 0.0
    cnt_tot = 0.0
    for r in results:
        m_tot += r["m_out"].astype(np.float64)
        p1_tot += float(r["misc_out"][:, 0].sum(dtype=np.float64))
        cnt_tot += float(r["misc_out"][:, 1].sum(dtype=np.float64))
    # sum_{c,j} W[j,c] * M[c,j]  (W symmetric)
    dot_tot = float((Wm * m_tot).sum())
    loss = (p1_tot - dot_tot) / (cnt_tot + EPS)
    return np.float32(loss)


def kernel(target, mask, pred_logit, bins_weight):
    target = np.asarray(target, dtype=np.float32)
    mask = np.asarray(mask)
    pred_logit = np.asarray(pred_logit, dtype=np.float32)
    bins_weight = np.asarray(bins_weight, dtype=np.float32)
    nc = build_nc()
    in_maps = shard_inputs(target, mask, pred_logit)
    res = bass_utils.run_bass_kernel_spmd(nc, in_maps,
                                          core_ids=list(range(N_CORES)))
    return combine(res.results, bins_weight)


if __name__ == "__main__":
    np.random.seed(0)
    tgt = (np.random.rand(B, 1, H, W) * 100).astype(np.float32)
    msk = np.random.rand(B, 1, H, W) > 0.1
    x = np.random.randn(B, BINS, H, W).astype(np.float32)
    idx = np.arange(BINS, dtype=np.float32)
    wb = np.exp(-ALPHA * (idx[None, :] - idx[:, None]) ** 2).astype(np.float32)
    print(kernel(tgt, msk, x, wb))
